# revision 2
# baseline (speedup 1.0000x reference)
"""AttentionFlow kernel for 8 TRN2 NeuronCores.

Sharding: data-parallel over batch B=8, one batch element per core, params
replicated. No collectives. Each core computes its full (C, 4D) output.

Design (per core):
- Everything runs in transposed [feature, context] layout; ctx^T, q^T-
  derived operands (qmod2 incl. the sc column, sq row) and the fp8
  DoubleRow-packed weights are prepared host-side, so the device does
  only plain coalesced DMAs (per-dispatch HWDGE cost ~0.65us makes DMA
  count matter), priority-ordered so phase 1 starts as soon as the first
  ctx^T chunk lands.
- Phase 1 (per 128-context tile, software-pipelined lag 2): sim matmul
  (with sc riding as column 128 of a width-130 rhs), free-dim softmax
  via EXP+accum_out, e[c]=exp(max+sc) batched per 4 tiles, H partials as
  closed per-tile accumulation groups, u matmuls per tile-pair, single
  bf16 evacuation of u.
- Phase 2: Q2C 1/Z via a ones-matmul partition broadcast; the Q2C block
  is folded into the ctx block weights (ctx @ (W0 + diag(h/Z) W3)).
- Phase 3: per tile 8 matmuls: u and u*ctx blocks in fp8(e4m3)
  DoubleRow (K=256/instruction), ctx block in bf16; fp8 of the ctx block
  would break the 2e-2 error budget (measured 0.039 vs 0.008 now).
"""

import numpy as np
import ml_dtypes

import concourse.bass as bass
import concourse.mybir as mybir
import concourse.tile as tile
from concourse import bacc
from concourse.bass_utils import run_bass_kernel_spmd
from concourse.masks import make_identity

B, C, Q, D = 8, 2048, 128, 256
F = 4 * D          # 1024
CT = C // 128      # 16 context tiles
FP32 = mybir.dt.float32
BF16 = mybir.dt.bfloat16
FP8 = mybir.dt.float8e4
EXP = mybir.ActivationFunctionType.Exp
DR = mybir.MatmulPerfMode.DoubleRow

_cached = {}


def build_nc():
    nc = bacc.Bacc(None, target_bir_lowering=False, debug=False)

    qcat_ext = nc.declare_dram_parameter("qcat", [128, 516], BF16,
                                         isOutput=False)
    ctxt_ext = nc.declare_dram_parameter("ctxt", [D, C], BF16, isOutput=False)
    ctx_ext = nc.declare_dram_parameter("ctx", [C, D], BF16, isOutput=False)
    sq_ext = nc.declare_dram_parameter("sqrow", [1, 130], BF16, isOutput=False)
    w2tb_ext = nc.declare_dram_parameter("w2tb", [4 * 128, F], BF16,
                                         isOutput=False)
    w2t8_ext = nc.declare_dram_parameter("w2t8", [128, 4 * F], FP8,
                                          isOutput=False)
    b2_ext = nc.declare_dram_parameter("b2", [1, F], FP32, isOutput=False)
    out_ext = nc.declare_dram_parameter("out", [C, F], FP32, isOutput=True)

    with tile.TileContext(nc) as tc:
        with (
            tc.tile_pool(name="persist", bufs=1) as persist,
            tc.tile_pool(name="p1", bufs=3) as p1,
            tc.tile_pool(name="p3", bufs=3) as p3,
        ):
            # ---------------- persistent tiles ----------------
            qcat = persist.tile([128, 516], BF16, name="qcat", tag="qcat")
            q_bf = qcat[:, 0:256]
            qmod2 = qcat[:, 256:516]
            sq_row = persist.tile([1, 130], BF16, name="sq_row", tag="sq_row")
            w2tb = persist.tile([128, 4, F], BF16, name="w2tb", tag="w2tb")
            w2t8 = persist.tile([128, 4, F], FP8, name="w2t8", tag="w2t8")
            b2_sb = persist.tile([128, F], FP32, name="b2_sb", tag="b2_sb")
            ctx2 = persist.tile([128, 2, C], BF16, name="ctx2", tag="ctx2")
            U2 = persist.tile([128, 2, C], BF16, name="U2", tag="U2")
            ctx_nat = persist.tile([128, CT, D], BF16, name="cnat", tag="cnat")
            ident = persist.tile([128, 128], BF16, name="ident", tag="ident")
            ones_mat = persist.tile([128, 128], BF16, name="ones_m", tag="ones_m")
            ones_row = persist.tile([1, 128], BF16, name="ones_r", tag="ones_r")
            te_coll = persist.tile([128, CT], FP32, name="te_coll", tag="te_coll")
            e_coll = persist.tile([128, CT], BF16, name="e_coll", tag="e_coll")
            h_sb = persist.tile([128, 2], FP32, name="h_sb", tag="h_sb")
            invZb = persist.tile([128, 1], FP32, name="invZb", tag="invZb")

            # ---- prologue DMAs: coalesced, priority-ordered on sync -------
            nc.sync.dma_start(qcat[:], qcat_ext[:, :])
            # ctx^T in 512-col chunks (both halves per chunk via 3D AP)
            for g in range(4):
                gs = slice(g * 512, (g + 1) * 512)
                nc.sync.dma_start(
                    ctx2[:, :, gs],
                    ctxt_ext[:, gs].rearrange("(h p) c -> p h c", h=2),
                )
                if g == 0:
                    nc.sync.dma_start(sq_row[:], sq_ext[:, :])
            # bulk loads after the phase-1-critical data
            nc.sync.dma_start(
                ctx_nat[:],
                ctx_ext[:, :].rearrange("(i p) d -> p i d", i=CT),
            )
            nc.sync.dma_start(
                w2tb[:],
                w2tb_ext[:, :].rearrange("(t p) f -> p t f", t=4),
            )
            nc.sync.dma_start(w2t8[:], w2t8_ext[:, :])
            nc.sync.dma_start(b2_sb[:], b2_ext[0:1, :].to_broadcast((128, F)))

            make_identity(nc, ident[:])
            nc.gpsimd.memset(ones_mat[:], 1.0)
            nc.gpsimd.memset(ones_row[:], 1.0)
            # warm the ACT exp table while DMAs run
            wexp = p1.tile([1, 1], FP32, name="wexp", tag="wexp")
            nc.scalar.activation(wexp[:], ones_row[0:1, 0:1], EXP)

            with tc.tile_pool(name="p1ps", bufs=1, space="PSUM") as p1ps:
                # per-tile H partials in cols (h*16 + i); each matmul is its
                # own closed accumulation group (start=stop=True).
                h_part = p1ps.tile([128, 32], FP32, name="h_part", tag="misc",
                                   bufs=2)

                # ---------------- phase 1: per c-tile C2Q attention --------
                a_tiles = [None] * CT
                aT_pair = [None, None]

                def stage_a(i):
                    cs = slice(i * 128, (i + 1) * 128)
                    sim_ps = p1ps.tile([128, 130], FP32, name=f"sim{i}",
                                       tag="sim", bufs=2)
                    for h in range(2):
                        nc.tensor.matmul(
                            sim_ps[:], ctx2[:, h, cs],
                            qmod2[:, h * 130:(h + 1) * 130],
                            start=(h == 0), stop=False,
                        )
                    nc.tensor.matmul(sim_ps[:], ones_row[:], sq_row[:],
                                     start=False, stop=True)
                    nm = p1.tile([128, 1], FP32, name=f"nm{i}", tag="nm",
                                 bufs=4)
                    nc.vector.reduce_max(
                        nm[:], sim_ps[:, 0:128],
                        axis=mybir.AxisListType.X, negate=True,
                    )
                    # te = sc - nm = sc + max  (e = exp(te), batched later)
                    nc.vector.tensor_tensor(
                        te_coll[:, i:i + 1], sim_ps[:, 128:129],
                        nm[:], mybir.AluOpType.subtract,
                    )
                    p_bf = p1.tile([128, 128], BF16, name=f"p{i}", tag="p")
                    se = p1.tile([128, 1], FP32, name=f"se{i}", tag="se")
                    nc.scalar.activation(
                        p_bf[:], sim_ps[:, 0:128], EXP,
                        bias=nm[:], scale=1.0, accum_out=se[:],
                    )
                    inv_se = p1.tile([128, 1], FP32, name=f"ise{i}", tag="ise")
                    nc.vector.reciprocal(inv_se[:], se[:])
                    a_bf = p1.tile([128, 128], BF16, name=f"a{i}", tag="a")
                    nc.scalar.mul(a_bf[:], p_bf[:], inv_se[:])
                    a_tiles[i] = a_bf

                def stage_b(i):
                    # transpose a into the pair buffer; u + U8/m28 per pair
                    par = i % 2
                    if par == 0:
                        aT_pair[0] = p1ps.tile([128, 256], BF16,
                                               name=f"aTp{i}", tag="aT",
                                               bufs=2)
                        aT_pair[1] = p1.tile([128, 256], BF16,
                                             name=f"aTs{i}", tag="aTs",
                                             bufs=2)
                    ps, sb = aT_pair
                    nc.tensor.transpose(ps[:, par * 128:(par + 1) * 128],
                                        a_tiles[i][:], ident[:])
                    nc.vector.tensor_copy(sb[:, par * 128:(par + 1) * 128],
                                          ps[:, par * 128:(par + 1) * 128])
                    if par == 1:
                        k = i // 2
                        u_ps = p1ps.tile([128, 2, 256], FP32, name=f"u{k}",
                                         tag="u", bufs=2)
                        for h in range(2):
                            nc.tensor.matmul(
                                u_ps[:, h], q_bf[:, h * 128:(h + 1) * 128],
                                sb[:], start=True, stop=True,
                            )
                        for t in range(2):
                            ts2 = slice((2 * k + t) * 128,
                                        (2 * k + t + 1) * 128)
                            nc.vector.tensor_copy(
                                U2[:, :, ts2],
                                u_ps[:, :, t * 128:(t + 1) * 128])

                def stage_e(i0):
                    # batched e = exp(te) for tiles i0..i0+3, then H partials
                    nc.scalar.activation(
                        e_coll[:, i0:i0 + 4], te_coll[:, i0:i0 + 4], EXP,
                    )
                    for i in range(i0, i0 + 4):
                        for h in range(2):
                            nc.tensor.matmul(
                                h_part[:, h * CT + i:h * CT + i + 1],
                                ctx_nat[:, i, h * 128:(h + 1) * 128],
                                e_coll[:, i:i + 1],
                                start=True, stop=True,
                            )

                for i in range(CT):
                    stage_a(i)
                    if i >= 2:
                        stage_b(i - 2)
                    if i % 4 == 3 and i >= 3:
                        stage_e(i - 3)
                stage_b(CT - 2)
                stage_b(CT - 1)

                # ------------- phase 2: Q2C normalization + W fold ---------
                zzf = p1.tile([128, 1], FP32, name="zzf", tag="zzf")
                nc.vector.reduce_sum(zzf[:], e_coll[:], axis=mybir.AxisListType.X)
                zzb = p1.tile([128, 1], BF16, name="zzb", tag="zzb")
                nc.vector.tensor_copy(zzb[:], zzf[:])
                zb_ps = p1ps.tile([128, 1], FP32, name="zb_ps", tag="misc",
                                  bufs=2)
                nc.tensor.matmul(zb_ps[:], ones_mat[:], zzb[:],
                                 start=True, stop=True)
                nc.vector.reciprocal(invZb[:], zb_ps[:])
                for h in range(2):
                    nc.vector.reduce_sum(h_sb[:, h:h + 1],
                                         h_part[:, h * CT:(h + 1) * CT],
                                         axis=mybir.AxisListType.X)

                # fold Q2C block into ctx block: w2tb[h] += h/Z * w2tb[2+h]
                for h in range(2):
                    wtmp = p1.tile([128, F], BF16, name=f"wtmp{h}", tag="wtmp")
                    nc.vector.tensor_scalar(
                        wtmp[:], w2tb[:, 2 + h], h_sb[:, h:h + 1],
                        invZb[:, 0:1],
                        mybir.AluOpType.mult, mybir.AluOpType.mult,
                    )
                    nc.vector.tensor_tensor(
                        w2tb[:, h], w2tb[:, h], wtmp[:], mybir.AluOpType.add
                    )

            # fp8 U8/m28 derived per tile (own pool tiles so phase-3 tile i
            # depends only on its own casts, not all 16)
            u8_t = []
            m2_t = []
            for i in range(CT):
                cs8 = slice(i * 128, (i + 1) * 128)
                u8 = p3.tile([128, 2, 128], FP8, name=f"u8_{i}",
                             tag=f"u8_{i % 16}", bufs=1)
                nc.scalar.copy(u8[:], U2[:, :, cs8])
                u8_t.append(u8)
                m2 = p3.tile([128, 2, 128], FP8, name=f"m2_{i}",
                             tag=f"m2_{i % 16}", bufs=1)
                nc.vector.tensor_mul(m2[:], U2[:, :, cs8], ctx2[:, :, cs8])
                m2_t.append(m2)

            # ---------------- phase 3: g = mega @ W2T + b2 ----------------
            with tc.tile_pool(name="p3ps", bufs=3, space="PSUM") as p3ps:
                for i in range(CT):
                    cs = slice(i * 128, (i + 1) * 128)
                    g_ps = [p3ps.tile([128, 512], FP32, name=f"g{j}_{i}",
                                      tag=f"g{j}", bufs=3) for j in range(2)]
                    for j in range(2):
                        js = slice(j * 512, (j + 1) * 512)
                        nc.tensor.matmul(
                            g_ps[j][:], u8_t[i][:], w2t8[:, 0:2, js],
                            start=True, stop=False, perf_mode=DR,
                        )
                        nc.tensor.matmul(
                            g_ps[j][:], m2_t[i][:], w2t8[:, 2:4, js],
                            start=False, stop=False, perf_mode=DR,
                        )
                        for h in range(2):
                            nc.tensor.matmul(
                                g_ps[j][:], ctx2[:, h, cs], w2tb[:, h, js],
                                start=False, stop=(h == 1),
                            )
                    g_sb = p3.tile([128, F], FP32, name=f"g_sb{i}", tag="g_sb")
                    if i >= CT - 2:
                        # tail tiles: per-half bias+DMA so output starts early
                        for j in range(2):
                            fs = slice(j * 512, (j + 1) * 512)
                            nc.vector.tensor_tensor(
                                g_sb[:, fs], g_ps[j][:], b2_sb[:, fs],
                                mybir.AluOpType.add,
                            )
                            nc.sync.dma_start(out_ext[cs, fs], g_sb[:, fs])
                    else:
                        for j in range(2):
                            fs = slice(j * 512, (j + 1) * 512)
                            nc.vector.tensor_tensor(
                                g_sb[:, fs], g_ps[j][:], b2_sb[:, fs],
                                mybir.AluOpType.add,
                            )
                        nc.sync.dma_start(out_ext[cs, :], g_sb[:])

    nc.finalize()
    return nc


def make_in_maps(inputs):
    """Build per-core input maps from full (unsharded) numpy inputs."""
    bf16 = ml_dtypes.bfloat16
    fp8 = ml_dtypes.float8_e4m3fn
    questions = np.asarray(inputs["questions"], dtype=np.float32)
    contexts = np.asarray(inputs["contexts"], dtype=np.float32)
    w_sim = np.asarray(inputs["w_sim"], dtype=np.float32)
    W2 = np.asarray(inputs["W2"], dtype=np.float32)
    w2t = np.ascontiguousarray(W2.T).astype(np.float32)   # [F(mega), F(out)]
    wc, wq, wcq = w_sim[:D], w_sim[D:2 * D], w_sim[2 * D:]
    w2tb = np.concatenate([w2t[0:2 * 128], w2t[6 * 128:8 * 128]], axis=0)
    w2t8u = np.ascontiguousarray(
        w2t[2 * 128:4 * 128].reshape(2, 128, F).transpose(1, 0, 2)
    ).reshape(128, 2 * F)
    w2t8m = np.ascontiguousarray(
        w2t[4 * 128:6 * 128].reshape(2, 128, F).transpose(1, 0, 2)
    ).reshape(128, 2 * F)
    b2f = np.asarray(inputs["b2"], dtype=np.float32).reshape(1, F)
    in_maps = []
    for i in range(B):
        qi = questions[i].astype(bf16)
        ci = contexts[i].astype(bf16)
        qT = qi.astype(np.float32).T                      # [D, Q]
        qmod = qT * wcq[:, None]
        qmod2 = np.zeros((128, 260), np.float32)
        for h in range(2):
            qmod2[:, h * 130:h * 130 + 128] = qmod[h * 128:(h + 1) * 128]
            qmod2[:, h * 130 + 128] = wc[h * 128:(h + 1) * 128]
        sqv = qi.astype(np.float32) @ wq.astype(np.float32)
        sq_row = np.zeros((1, 130), np.float32)
        sq_row[0, :128] = sqv
        qcat = np.concatenate([qi.astype(np.float32), qmod2], axis=1)
        in_maps.append({
            "sqrow": sq_row.astype(bf16),
            "qcat": qcat.astype(bf16),
            "ctx": ci,
            "ctxt": np.ascontiguousarray(ci.T),
            "w2tb": w2tb.astype(bf16),
            "w2t8": np.clip(np.concatenate([w2t8u, w2t8m], axis=1),
                            -240, 240).astype(fp8),
            "b2": b2f,
        })
    return in_maps


def kernel(questions, contexts, questions_mask, contexts_mask, w_sim, W2, b2):
    if "nc" not in _cached:
        _cached["nc"] = build_nc()
    nc = _cached["nc"]
    in_maps = make_in_maps({
        "questions": questions, "contexts": contexts,
        "w_sim": w_sim, "W2": W2, "b2": b2,
    })
    res = run_bass_kernel_spmd(nc, in_maps, core_ids=list(range(B)))
    out = np.stack([res.results[i]["out"] for i in range(B)], axis=0)
    return out.astype(np.float32)
